# revision 3
# baseline (speedup 1.0000x reference)
"""DeepSet GNN message-passing kernel for 8 TRN2 NeuronCores.

Strategy:
  - segment_ids are sorted, so shard by *segment windows*: 392 windows of 128
    segments, 49 windows per core. Each core handles exactly the neighbor rows
    whose segment falls in its windows -> no cross-core reduction at all.
  - Host folds phi_w2 past the segment sum (segment_sum(h@W2+b2) =
    segment_sum(h)@W2 + counts*b2), transposes neighbors to fp16 [64, N] and
    pads each window's rows to a fixed B blocks of 128 rows so all 8 cores run
    one identical (SPMD) program.
  - Device per 128-row block: self-loading matmul h1 = relu-able (X_aug @ W1_aug)
    into PSUM, ACT relu-copy to fp16, DVE builds a one-hot [row, seg] via
    iota/is_equal against the row's local segment id, PE accumulates
    one_hot.T @ h1 into a per-window PSUM tile [128 segs, 64].
  - Per window: tiny rho MLP chain on PE/ACT (transpose via PE identity,
    biases and the counts*b2 term folded in via augmented rows), output
    written transposed [2, segs]; host re-transposes.
"""

import os
import sys

sys.path.insert(0, "/opt/trn_rl_repo")

import numpy as np

last_results = None  # stash of BassKernelResults when KERNEL_TRACE=1

N_AGENTS = 50000
N_NEIGH = 1600000
D = 64
N_CORES = 8
SEG_T = 128  # segments per window (= PSUM partition dim)
W_PER_CORE = 49
NW_TOT = N_CORES * W_PER_CORE  # 392 windows of 128 segs >= 50000
SEGS_PER_CORE = W_PER_CORE * SEG_T  # 6272


def _build_program(B):
    """Build the SPMD bacc program for B blocks (of 128 rows) per window."""
    from concourse import bacc, mybir
    import concourse.tile as tile

    FP16 = mybir.dt.float16
    F32 = mybir.dt.float32
    Relu = mybir.ActivationFunctionType.Relu
    Copy = mybir.ActivationFunctionType.Copy

    NBLK = W_PER_CORE * B
    NCOL = NBLK * 128

    nc = bacc.Bacc("TRN2", target_bir_lowering=False, debug=False)
    xta = nc.dram_tensor("xta", [65, NCOL], FP16, kind="ExternalInput").ap()
    qid = nc.dram_tensor("qid", [128, NBLK], F32, kind="ExternalInput").ap()
    cnt = nc.dram_tensor("cnt", [2, SEGS_PER_CORE], FP16, kind="ExternalInput").ap()
    w1a = nc.dram_tensor("w1a", [65, 64], FP16, kind="ExternalInput").ap()
    waa = nc.dram_tensor("waa", [66, 64], FP16, kind="ExternalInput").ap()
    wba = nc.dram_tensor("wba", [65, 2], FP16, kind="ExternalInput").ap()
    iota = nc.dram_tensor("iota", [128, 128], FP16, kind="ExternalInput").ap()
    iden = nc.dram_tensor("iden", [128, 128], FP16, kind="ExternalInput").ap()
    out = nc.dram_tensor("out", [2, SEGS_PER_CORE], F32, kind="ExternalOutput").ap()

    with tile.TileContext(nc) as tc:
        with (
            tc.tile_pool(name="const", bufs=1) as cpool,
            tc.tile_pool(name="x", bufs=6) as xpool,
            tc.tile_pool(name="h", bufs=4) as hpool,
            tc.tile_pool(name="oh", bufs=4) as ohpool,
            tc.tile_pool(name="rho", bufs=2) as rpool,
            tc.tile_pool(name="psh", bufs=3, space="PSUM") as psh,
            tc.tile_pool(name="pss", bufs=2, space="PSUM") as pss,
            tc.tile_pool(name="pst", bufs=1, space="PSUM") as pst,
            tc.tile_pool(name="psr", bufs=1, space="PSUM") as psr,
            tc.tile_pool(name="pso", bufs=1, space="PSUM") as pso,
        ):
            w1a_t = cpool.tile([65, 64], FP16)
            nc.sync.dma_start(w1a_t[:], w1a[:, :])
            waa_t = cpool.tile([66, 64], FP16)
            nc.sync.dma_start(waa_t[:], waa[:, :])
            wba_t = cpool.tile([65, 2], FP16)
            nc.sync.dma_start(wba_t[:], wba[:, :])
            iota_t = cpool.tile([128, 128], FP16)
            nc.sync.dma_start(iota_t[:], iota[:, :])
            iden_t = cpool.tile([128, 128], FP16)
            nc.sync.dma_start(iden_t[:], iden[:, :])
            # all per-block segment ids, loaded once: [128, NBLK] f32
            qall_t = cpool.tile([128, NBLK], F32)
            nc.sync.dma_start(qall_t[:], qid[:, :])

            for w in range(W_PER_CORE):
                s_ps = pss.tile([128, 64], F32)
                for j in range(0, B, 2):
                    col0 = 128 * (B * w + j)
                    xt = xpool.tile([65, 256], FP16)
                    nc.sync.dma_start(xt[:], xta[:, col0 : col0 + 256])
                    for k in range(2):
                        b = j + k
                        hp = psh.tile([128, 64], F32)
                        nc.tensor.matmul(
                            hp[:], lhsT=xt[:, 128 * k : 128 * k + 128],
                            rhs=w1a_t[:], start=True, stop=True,
                        )
                        hs = hpool.tile([128, 64], FP16)
                        nc.scalar.activation(hs[:], hp[:], Relu)
                        oh = ohpool.tile([128, 128], FP16)
                        nc.vector.tensor_scalar(
                            out=oh[:], in0=iota_t[:],
                            scalar1=qall_t[:, B * w + b : B * w + b + 1],
                            scalar2=0.0,
                            op0=mybir.AluOpType.subtract,
                            op1=mybir.AluOpType.is_equal,
                        )
                        nc.tensor.matmul(
                            s_ps[:], lhsT=oh[:], rhs=hs[:],
                            start=(b == 0), stop=(b == B - 1),
                        )
                # rho MLP on the window's 128 pooled segments
                s_sb = rpool.tile([128, 64], FP16)
                nc.scalar.activation(s_sb[:], s_ps[:], Copy)
                st_ps = pst.tile([64, 128], FP16)
                nc.tensor.transpose(st_ps[:], s_sb[:], iden_t[:])
                st_sb = rpool.tile([66, 128], FP16)
                nc.vector.tensor_copy(st_sb[0:64, :], st_ps[:])
                nc.sync.dma_start(
                    st_sb[64:66, :], cnt[:, SEG_T * w : SEG_T * w + SEG_T]
                )
                r_ps = psr.tile([64, 128], F32)
                nc.tensor.matmul(r_ps[:], lhsT=waa_t[:], rhs=st_sb[:], start=True, stop=True)
                r_sb = rpool.tile([65, 128], FP16)
                nc.scalar.activation(r_sb[0:64, :], r_ps[:], Relu)
                nc.sync.dma_start(
                    r_sb[64:65, :], cnt[1:2, SEG_T * w : SEG_T * w + SEG_T]
                )
                o_ps = pso.tile([2, 128], F32)
                nc.tensor.matmul(o_ps[:], lhsT=wba_t[:], rhs=r_sb[:], start=True, stop=True)
                o_sb = rpool.tile([2, 128], F32)
                nc.vector.tensor_copy(o_sb[:], o_ps[:])
                nc.sync.dma_start(out[:, SEG_T * w : SEG_T * w + SEG_T], o_sb[:])
    nc.compile()
    return nc


def _host_prep(neighbors, phi_w1, phi_b1, phi_w2, phi_b2,
               rho_w1, rho_b1, rho_w2, rho_b2, segment_ids):
    ids = np.asarray(segment_ids)
    X = np.asarray(neighbors)

    bounds = np.minimum(np.arange(NW_TOT + 1) * SEG_T, N_AGENTS)
    edges = np.searchsorted(ids, bounds)  # row range per window
    rows_w = np.diff(edges)
    B = int(np.ceil(rows_w.max() / 128))
    B += B % 2  # even so we can DMA two blocks at a time

    NBLK = W_PER_CORE * B
    NCOL = NBLK * 128

    XT = np.ascontiguousarray(X.T).astype(np.float16)  # [64, N]
    counts = np.bincount(ids, minlength=NW_TOT * SEG_T).astype(np.float16)

    in_maps = []
    consts = dict(
        w1a=np.concatenate([phi_w1, phi_b1[None, :]], 0).astype(np.float16),
        waa=np.concatenate(
            [phi_w2 @ rho_w1, (phi_b2 @ rho_w1)[None, :], rho_b1[None, :]], 0
        ).astype(np.float16),
        wba=np.concatenate([rho_w2, rho_b2[None, :]], 0).astype(np.float16),
        iota=np.tile(np.arange(128, dtype=np.float16), (128, 1)),
        iden=np.eye(128, dtype=np.float16),
    )
    for c in range(N_CORES):
        xta = np.zeros((65, NCOL), np.float16)
        qflat = np.full(NCOL, -1.0, np.float32)
        for wl in range(W_PER_CORE):
            wg = W_PER_CORE * c + wl
            a, e = edges[wg], edges[wg + 1]
            n = e - a
            c0 = wl * B * 128
            xta[0:64, c0 : c0 + n] = XT[:, a:e]
            xta[64, c0 : c0 + n] = 1.0
            qflat[c0 : c0 + n] = (ids[a:e] - SEG_T * wg).astype(np.float32)
        qid = np.ascontiguousarray(qflat.reshape(NBLK, 128).T)  # [128, NBLK]
        cnt = np.empty((2, SEGS_PER_CORE), np.float16)
        cnt[0] = counts[SEGS_PER_CORE * c : SEGS_PER_CORE * (c + 1)]
        cnt[1] = 1.0
        in_maps.append(dict(xta=xta, qid=qid, cnt=cnt, **consts))
    return B, in_maps


def kernel(**inputs):
    global last_results
    np_inputs = {k: np.asarray(v) for k, v in inputs.items()}
    B, in_maps = _host_prep(**np_inputs)
    nc = _build_program(B)

    from concourse.bass_utils import run_bass_kernel_spmd

    trace = bool(os.environ.get("KERNEL_TRACE"))
    res = run_bass_kernel_spmd(nc, in_maps, list(range(N_CORES)), trace=trace)
    if trace:
        last_results = res
    out_t = np.concatenate(
        [res.results[c]["out"] for c in range(N_CORES)], axis=1
    )  # [2, 50176]
    return np.ascontiguousarray(out_t[:, :N_AGENTS].T).astype(np.float32)



# revision 5
# speedup vs baseline: 2.2146x; 2.2146x over previous
"""DeepSet GNN message-passing kernel for 8 TRN2 NeuronCores.

Strategy:
  - segment_ids are sorted, so shard by *segment windows*: 392 windows of 128
    segments, 49 windows per core. Each core handles exactly the neighbor rows
    whose segment falls in its windows -> no cross-core reduction at all.
  - Host transposes neighbors to bf16 [65, N] (row 64 = ones) and pads each
    window's rows to B blocks of 128 so all 8 cores run one identical (SPMD)
    program. One large DMA per window.
  - w1a is [65, 65]: [phi_w1; phi_b1] plus a unit column so hs col 64 is 1.0
    for real rows / 0.0 for padding -> the pooling matmul accumulates per-
    segment counts for free in s_ps[:, 64].
  - Device per 128-row block: matmul h = relu(X_aug @ w1a) into a 7-block
    PSUM tile (one batched relu per 7 blocks), DVE builds a one-hot
    [row, seg] via iota/is_equal, PE accumulates one_hot.T @ h into a
    per-window PSUM tile [128 segs, 65].
  - Per window: PE transpose, rho MLP with per-partition bias APs
    (rho_b1/rho_b2), result written into a [2, 6272] SBUF tile; ONE output
    DMA at the end.
"""

import os
import sys

sys.path.insert(0, "/opt/trn_rl_repo")

import numpy as np

last_results = None  # stash of BassKernelResults when KERNEL_TRACE=1

N_AGENTS = 50000
N_NEIGH = 1600000
D = 64
N_CORES = 8
SEG_T = 128  # segments per window (= PSUM partition dim)
W_PER_CORE = 49
NW_TOT = N_CORES * W_PER_CORE  # 392 windows of 128 segs >= 50000
SEGS_PER_CORE = W_PER_CORE * SEG_T  # 6272
RELU_BATCH = 7  # blocks per batched relu; 7*65 = 455 f32 <= one PSUM bank


def _build_program(B):
    """Build the SPMD bacc program for B blocks (of 128 rows) per window."""
    from concourse import bacc, mybir
    import concourse.tile as tile

    BF16 = mybir.dt.bfloat16
    F32 = mybir.dt.float32
    Relu = mybir.ActivationFunctionType.Relu
    Ident = mybir.ActivationFunctionType.Identity
    Copy = mybir.ActivationFunctionType.Copy

    NBLK = W_PER_CORE * B
    NCOL = NBLK * 128
    WCOL = B * 128  # columns per window

    nc = bacc.Bacc("TRN2", target_bir_lowering=False, debug=False)
    xta = nc.dram_tensor("xta", [65, NCOL], BF16, kind="ExternalInput").ap()
    qid = nc.dram_tensor("qid", [128, NBLK], F32, kind="ExternalInput").ap()
    w1a = nc.dram_tensor("w1a", [65, 65], BF16, kind="ExternalInput").ap()
    waa = nc.dram_tensor("waa", [65, 64], BF16, kind="ExternalInput").ap()
    wba = nc.dram_tensor("wba", [64, 2], BF16, kind="ExternalInput").ap()
    rb1 = nc.dram_tensor("rb1", [64, 1], F32, kind="ExternalInput").ap()
    rb2 = nc.dram_tensor("rb2", [2, 1], F32, kind="ExternalInput").ap()
    iota = nc.dram_tensor("iota", [128, 128], BF16, kind="ExternalInput").ap()
    iden = nc.dram_tensor("iden", [128, 128], BF16, kind="ExternalInput").ap()
    out = nc.dram_tensor("out", [2, SEGS_PER_CORE], F32, kind="ExternalOutput").ap()

    with tile.TileContext(nc) as tc:
        with (
            tc.tile_pool(name="const", bufs=1) as cpool,
            tc.tile_pool(name="x", bufs=3) as xpool,
            tc.tile_pool(name="h", bufs=3) as hpool,
            tc.tile_pool(name="oh", bufs=4) as ohpool,
            tc.tile_pool(name="rho", bufs=2) as rpool,
            tc.tile_pool(name="psh", bufs=2, space="PSUM") as psh,
            tc.tile_pool(name="pss", bufs=2, space="PSUM") as pss,
            tc.tile_pool(name="pst", bufs=1, space="PSUM") as pst,
            tc.tile_pool(name="psr", bufs=1, space="PSUM") as psr,
            tc.tile_pool(name="pso", bufs=1, space="PSUM") as pso,
        ):
            w1a_t = cpool.tile([65, 65], BF16)
            nc.sync.dma_start(w1a_t[:], w1a[:, :])
            waa_t = cpool.tile([65, 64], BF16)
            nc.sync.dma_start(waa_t[:], waa[:, :])
            wba_t = cpool.tile([64, 2], BF16)
            nc.sync.dma_start(wba_t[:], wba[:, :])
            rb1_t = cpool.tile([64, 1], F32)
            nc.sync.dma_start(rb1_t[:], rb1[:, :])
            rb2_t = cpool.tile([2, 1], F32)
            nc.sync.dma_start(rb2_t[:], rb2[:, :])
            iota_t = cpool.tile([128, 128], BF16)
            nc.sync.dma_start(iota_t[:], iota[:, :])
            iden_t = cpool.tile([128, 128], BF16)
            nc.sync.dma_start(iden_t[:], iden[:, :])
            # all per-block segment ids, loaded once: [128, NBLK] f32
            qall_t = cpool.tile([128, NBLK], F32)
            nc.sync.dma_start(qall_t[:], qid[:, :])
            # final output accumulates here; single DMA at the end
            out_t = cpool.tile([2, SEGS_PER_CORE], F32)

            for w in range(W_PER_CORE):
                # one big DMA per window
                xt = xpool.tile([65, WCOL], BF16)
                nc.sync.dma_start(xt[:], xta[:, WCOL * w : WCOL * (w + 1)])

                s_ps = pss.tile([128, 65], F32)
                b = 0
                while b < B:
                    nb = min(RELU_BATCH, B - b)
                    hp = psh.tile([128, RELU_BATCH * 65], F32)
                    for k in range(nb):
                        nc.tensor.matmul(
                            hp[:, 65 * k : 65 * k + 65],
                            lhsT=xt[:, 128 * (b + k) : 128 * (b + k) + 128],
                            rhs=w1a_t[:],
                            start=True,
                            stop=True,
                        )
                    hs = hpool.tile([128, RELU_BATCH * 65], BF16)
                    nc.scalar.activation(
                        hs[:, : 65 * nb], hp[:, : 65 * nb], Relu
                    )
                    for k in range(nb):
                        oh = ohpool.tile([128, 128], BF16)
                        nc.vector.tensor_scalar(
                            out=oh[:],
                            in0=iota_t[:],
                            scalar1=qall_t[:, B * w + b + k : B * w + b + k + 1],
                            scalar2=0.0,
                            op0=mybir.AluOpType.subtract,
                            op1=mybir.AluOpType.is_equal,
                        )
                        nc.tensor.matmul(
                            s_ps[:],
                            lhsT=oh[:],
                            rhs=hs[:, 65 * k : 65 * k + 65],
                            start=(b + k == 0),
                            stop=(b + k == B - 1),
                        )
                    b += nb
                # rho MLP on the window's 128 pooled segments
                s_sb = rpool.tile([128, 65], BF16)
                nc.scalar.activation(s_sb[:], s_ps[:], Copy)
                st_ps = pst.tile([65, 128], BF16)
                nc.tensor.transpose(st_ps[:], s_sb[:], iden_t[:])
                st_sb = rpool.tile([65, 128], BF16)
                nc.vector.tensor_copy(st_sb[:], st_ps[:])
                r_ps = psr.tile([64, 128], F32)
                nc.tensor.matmul(r_ps[:], lhsT=waa_t[:], rhs=st_sb[:], start=True, stop=True)
                r_sb = rpool.tile([64, 128], BF16)
                nc.scalar.activation(r_sb[:], r_ps[:], Relu, bias=rb1_t[:])
                o_ps = pso.tile([2, 128], F32)
                nc.tensor.matmul(o_ps[:], lhsT=wba_t[:], rhs=r_sb[:], start=True, stop=True)
                nc.scalar.activation(
                    out_t[:, SEG_T * w : SEG_T * (w + 1)], o_ps[:], Ident,
                    bias=rb2_t[:],
                )
            nc.sync.dma_start(out[:, :], out_t[:])
    nc.compile()
    return nc


def _host_prep(neighbors, phi_w1, phi_b1, phi_w2, phi_b2,
               rho_w1, rho_b1, rho_w2, rho_b2, segment_ids):
    ids = np.asarray(segment_ids)
    X = np.asarray(neighbors)

    bounds = np.minimum(np.arange(NW_TOT + 1) * SEG_T, N_AGENTS)
    edges = np.searchsorted(ids, bounds)  # row range per window
    rows_w = np.diff(edges)
    B = int(np.ceil(rows_w.max() / 128))

    NBLK = W_PER_CORE * B
    NCOL = NBLK * 128

    import ml_dtypes

    BF = ml_dtypes.bfloat16
    XT = np.ascontiguousarray(X.T).astype(BF)  # [64, N]

    w1a = np.zeros((65, 65), np.float32)
    w1a[0:64, 0:64] = phi_w1
    w1a[64, 0:64] = phi_b1
    w1a[64, 64] = 1.0

    in_maps = []
    consts = dict(
        w1a=w1a.astype(BF),
        waa=np.concatenate(
            [phi_w2 @ rho_w1, (phi_b2 @ rho_w1)[None, :]], 0
        ).astype(BF),
        wba=np.asarray(rho_w2).astype(BF),
        rb1=np.asarray(rho_b1).reshape(64, 1).astype(np.float32),
        rb2=np.asarray(rho_b2).reshape(2, 1).astype(np.float32),
        iota=np.tile(np.arange(128, dtype=np.float32), (128, 1)).astype(BF),
        iden=np.eye(128, dtype=np.float32).astype(BF),
    )
    for c in range(N_CORES):
        xta = np.zeros((65, NCOL), BF)
        qflat = np.full(NCOL, -1.0, np.float32)
        for wl in range(W_PER_CORE):
            wg = W_PER_CORE * c + wl
            a, e = edges[wg], edges[wg + 1]
            n = e - a
            c0 = wl * B * 128
            xta[0:64, c0 : c0 + n] = XT[:, a:e]
            xta[64, c0 : c0 + n] = 1.0
            qflat[c0 : c0 + n] = (ids[a:e] - SEG_T * wg).astype(np.float32)
        qid = np.ascontiguousarray(qflat.reshape(NBLK, 128).T)
        in_maps.append(dict(xta=xta, qid=qid, **consts))
    return B, in_maps


def kernel(**inputs):
    global last_results
    np_inputs = {k: np.asarray(v) for k, v in inputs.items()}
    B, in_maps = _host_prep(**np_inputs)
    nc = _build_program(B)

    from concourse.bass_utils import run_bass_kernel_spmd

    trace = bool(os.environ.get("KERNEL_TRACE"))
    res = run_bass_kernel_spmd(nc, in_maps, list(range(N_CORES)), trace=trace)
    if trace:
        last_results = res
    out_t = np.concatenate(
        [res.results[c]["out"] for c in range(N_CORES)], axis=1
    )  # [2, 50176]
    return np.ascontiguousarray(out_t[:, :N_AGENTS].T).astype(np.float32)


# revision 6
# speedup vs baseline: 2.7783x; 1.2545x over previous
"""DeepSet GNN message-passing kernel for 8 TRN2 NeuronCores.

Strategy:
  - segment_ids are sorted, so shard by *segment windows*: 392 windows of 128
    segments, 49 windows per core. Each core handles exactly the neighbor rows
    whose segment falls in its windows -> no cross-core reduction at all.
  - Host transposes neighbors to bf16 [65, N] (row 64 = ones) and pads each
    window's rows to B blocks of 128 so all 8 cores run one identical (SPMD)
    program. One large DMA per window.
  - w1a is [65, 65]: [phi_w1; phi_b1] plus a unit column so hs col 64 is 1.0
    for real rows / 0.0 for padding -> the pooling matmul accumulates per-
    segment counts for free in s_ps[:, 64].
  - Device per 128-row block: matmul h = relu(X_aug @ w1a) into a 7-block
    PSUM tile (one batched relu per 7 blocks), DVE builds a one-hot
    [row, seg] via iota/is_equal, PE accumulates one_hot.T @ h into a
    per-window PSUM tile [128 segs, 65].
  - Per window: PE transpose, rho MLP with per-partition bias APs
    (rho_b1/rho_b2), result written into a [2, 6272] SBUF tile; ONE output
    DMA at the end.
"""

import os
import sys

sys.path.insert(0, "/opt/trn_rl_repo")

import numpy as np

last_results = None  # stash of BassKernelResults when KERNEL_TRACE=1

N_AGENTS = 50000
N_NEIGH = 1600000
D = 64
N_CORES = 8
SEG_T = 128  # segments per window (= PSUM partition dim)
W_PER_CORE = 49
NW_TOT = N_CORES * W_PER_CORE  # 392 windows of 128 segs >= 50000
SEGS_PER_CORE = W_PER_CORE * SEG_T  # 6272
RELU_BATCH = 7  # blocks per batched relu; 7*65 = 455 f32 <= one PSUM bank


def _build_program(B):
    """Build the SPMD bacc program for B blocks (of 128 rows) per window."""
    from concourse import bacc, mybir
    import concourse.tile as tile

    BF16 = mybir.dt.bfloat16
    FP16 = mybir.dt.float16
    F32 = mybir.dt.float32
    Relu = mybir.ActivationFunctionType.Relu
    Ident = mybir.ActivationFunctionType.Identity
    Copy = mybir.ActivationFunctionType.Copy

    NBLK = W_PER_CORE * B
    NCOL = NBLK * 128
    WCOL = B * 128  # columns per window

    nc = bacc.Bacc("TRN2", target_bir_lowering=False, debug=False)
    xta = nc.dram_tensor("xta", [65, NCOL], BF16, kind="ExternalInput").ap()
    qid = nc.dram_tensor("qid", [128, NBLK], F32, kind="ExternalInput").ap()
    w1a = nc.dram_tensor("w1a", [65, 65], BF16, kind="ExternalInput").ap()
    waa = nc.dram_tensor("waa", [65, 64], FP16, kind="ExternalInput").ap()
    wba = nc.dram_tensor("wba", [64, 2], FP16, kind="ExternalInput").ap()
    rb1 = nc.dram_tensor("rb1", [64, 1], F32, kind="ExternalInput").ap()
    rb2 = nc.dram_tensor("rb2", [2, 1], F32, kind="ExternalInput").ap()
    iota = nc.dram_tensor("iota", [128, 128], BF16, kind="ExternalInput").ap()
    iden = nc.dram_tensor("iden", [128, 128], FP16, kind="ExternalInput").ap()
    out = nc.dram_tensor("out", [2, SEGS_PER_CORE], F32, kind="ExternalOutput").ap()

    with tile.TileContext(nc) as tc:
        with (
            tc.tile_pool(name="const", bufs=1) as cpool,
            tc.tile_pool(name="x", bufs=3) as xpool,
            tc.tile_pool(name="h", bufs=4) as hpool,
            tc.tile_pool(name="oh", bufs=8) as ohpool,
            tc.tile_pool(name="rho", bufs=2) as rpool,
            tc.tile_pool(name="psh", bufs=2, space="PSUM") as psh,
            tc.tile_pool(name="pss", bufs=2, space="PSUM") as pss,
            tc.tile_pool(name="pst", bufs=1, space="PSUM") as pst,
            tc.tile_pool(name="psr", bufs=1, space="PSUM") as psr,
            tc.tile_pool(name="pso", bufs=1, space="PSUM") as pso,
        ):
            w1a_t = cpool.tile([65, 65], BF16)
            nc.sync.dma_start(w1a_t[:], w1a[:, :])
            waa_t = cpool.tile([65, 64], FP16)
            nc.sync.dma_start(waa_t[:], waa[:, :])
            wba_t = cpool.tile([64, 2], FP16)
            nc.sync.dma_start(wba_t[:], wba[:, :])
            rb1_t = cpool.tile([64, 1], F32)
            nc.sync.dma_start(rb1_t[:], rb1[:, :])
            rb2_t = cpool.tile([2, 1], F32)
            nc.sync.dma_start(rb2_t[:], rb2[:, :])
            iota_t = cpool.tile([128, 128], BF16)
            nc.sync.dma_start(iota_t[:], iota[:, :])
            iden_t = cpool.tile([128, 128], FP16)
            nc.sync.dma_start(iden_t[:], iden[:, :])
            # all per-block segment ids, loaded once: [128, NBLK] f32
            qall_t = cpool.tile([128, NBLK], F32)
            nc.sync.dma_start(qall_t[:], qid[:, :])
            # final output accumulates here; single DMA at the end
            out_t = cpool.tile([2, SEGS_PER_CORE], F32)

            for w in range(W_PER_CORE):
                # one big DMA per window
                xt = xpool.tile([65, WCOL], BF16)
                nc.sync.dma_start(xt[:], xta[:, WCOL * w : WCOL * (w + 1)])

                s_ps = pss.tile([128, 65], F32)
                b = 0
                while b < B:
                    nb = min(RELU_BATCH, B - b)
                    hp = psh.tile([128, RELU_BATCH * 65], F32)
                    for k in range(nb):
                        nc.tensor.matmul(
                            hp[:, 65 * k : 65 * k + 65],
                            lhsT=xt[:, 128 * (b + k) : 128 * (b + k) + 128],
                            rhs=w1a_t[:],
                            start=True,
                            stop=True,
                        )
                    hs = hpool.tile([128, RELU_BATCH * 65], BF16)
                    nc.scalar.activation(
                        hs[:, : 65 * nb], hp[:, : 65 * nb], Relu
                    )
                    for k in range(nb):
                        oh = ohpool.tile([128, 128], BF16)
                        nc.vector.tensor_scalar(
                            out=oh[:],
                            in0=iota_t[:],
                            scalar1=qall_t[:, B * w + b + k : B * w + b + k + 1],
                            scalar2=None,
                            op0=mybir.AluOpType.is_equal,
                        )
                        nc.tensor.matmul(
                            s_ps[:],
                            lhsT=oh[:],
                            rhs=hs[:, 65 * k : 65 * k + 65],
                            start=(b + k == 0),
                            stop=(b + k == B - 1),
                        )
                    b += nb
                # rho MLP on the window's 128 pooled segments
                s_sb = rpool.tile([128, 65], FP16)
                nc.scalar.activation(s_sb[:], s_ps[:], Copy)
                st_ps = pst.tile([65, 128], FP16)
                nc.tensor.transpose(st_ps[:], s_sb[:], iden_t[:])
                st_sb = rpool.tile([65, 128], FP16)
                nc.vector.tensor_copy(st_sb[:], st_ps[:])
                r_ps = psr.tile([64, 128], F32)
                nc.tensor.matmul(r_ps[:], lhsT=waa_t[:], rhs=st_sb[:], start=True, stop=True)
                r_sb = rpool.tile([64, 128], FP16)
                nc.scalar.activation(r_sb[:], r_ps[:], Relu, bias=rb1_t[:])
                o_ps = pso.tile([2, 128], F32)
                nc.tensor.matmul(o_ps[:], lhsT=wba_t[:], rhs=r_sb[:], start=True, stop=True)
                nc.scalar.activation(
                    out_t[:, SEG_T * w : SEG_T * (w + 1)], o_ps[:], Ident,
                    bias=rb2_t[:],
                )
            nc.sync.dma_start(out[:, :], out_t[:])
    nc.compile()
    return nc


def _host_prep(neighbors, phi_w1, phi_b1, phi_w2, phi_b2,
               rho_w1, rho_b1, rho_w2, rho_b2, segment_ids):
    ids = np.asarray(segment_ids)
    X = np.asarray(neighbors)

    bounds = np.minimum(np.arange(NW_TOT + 1) * SEG_T, N_AGENTS)
    edges = np.searchsorted(ids, bounds)  # row range per window
    rows_w = np.diff(edges)
    B = int(np.ceil(rows_w.max() / 128))

    NBLK = W_PER_CORE * B
    NCOL = NBLK * 128

    import ml_dtypes

    BF = ml_dtypes.bfloat16
    XT = np.ascontiguousarray(X.T).astype(BF)  # [64, N]

    w1a = np.zeros((65, 65), np.float32)
    w1a[0:64, 0:64] = phi_w1
    w1a[64, 0:64] = phi_b1
    w1a[64, 64] = 1.0

    in_maps = []
    consts = dict(
        w1a=w1a.astype(BF),
        waa=np.concatenate(
            [phi_w2 @ rho_w1, (phi_b2 @ rho_w1)[None, :]], 0
        ).astype(np.float16),
        wba=np.asarray(rho_w2).astype(np.float16),
        rb1=np.asarray(rho_b1).reshape(64, 1).astype(np.float32),
        rb2=np.asarray(rho_b2).reshape(2, 1).astype(np.float32),
        iota=np.tile(np.arange(128, dtype=np.float32), (128, 1)).astype(BF),
        iden=np.eye(128, dtype=np.float16),
    )
    for c in range(N_CORES):
        xta = np.zeros((65, NCOL), BF)
        qflat = np.full(NCOL, -1.0, np.float32)
        for wl in range(W_PER_CORE):
            wg = W_PER_CORE * c + wl
            a, e = edges[wg], edges[wg + 1]
            n = e - a
            c0 = wl * B * 128
            xta[0:64, c0 : c0 + n] = XT[:, a:e]
            xta[64, c0 : c0 + n] = 1.0
            qflat[c0 : c0 + n] = (ids[a:e] - SEG_T * wg).astype(np.float32)
        qid = np.ascontiguousarray(qflat.reshape(NBLK, 128).T)
        in_maps.append(dict(xta=xta, qid=qid, **consts))
    return B, in_maps


def kernel(**inputs):
    global last_results
    np_inputs = {k: np.asarray(v) for k, v in inputs.items()}
    B, in_maps = _host_prep(**np_inputs)
    nc = _build_program(B)

    from concourse.bass_utils import run_bass_kernel_spmd

    trace = bool(os.environ.get("KERNEL_TRACE"))
    res = run_bass_kernel_spmd(nc, in_maps, list(range(N_CORES)), trace=trace)
    if trace:
        last_results = res
    out_t = np.concatenate(
        [res.results[c]["out"] for c in range(N_CORES)], axis=1
    )  # [2, 50176]
    return np.ascontiguousarray(out_t[:, :N_AGENTS].T).astype(np.float32)


# revision 7
# speedup vs baseline: 3.4762x; 1.2512x over previous
"""DeepSet GNN message-passing kernel for 8 TRN2 NeuronCores.

Strategy:
  - segment_ids are sorted, so shard by *segment windows*: 392 windows of 128
    segments, 49 windows per core. Each core handles exactly the neighbor rows
    whose segment falls in its windows -> no cross-core reduction at all.
  - Host transposes neighbors to bf16 [65, N] (row 64 = ones) and pads each
    window's rows to B blocks of 128 so all 8 cores run one identical (SPMD)
    program. One large DMA per window.
  - w1a is [65, 65]: [phi_w1; phi_b1] plus a unit column so hs col 64 is 1.0
    for real rows / 0.0 for padding -> the pooling matmul accumulates per-
    segment counts for free in s_ps[:, 64].
  - Device per 128-row block: matmul h = relu(X_aug @ w1a) into a 7-block
    PSUM tile (one batched relu per 7 blocks), DVE builds a one-hot
    [row, seg] via iota/is_equal, PE accumulates one_hot.T @ h into a
    per-window PSUM tile [128 segs, 65].
  - Per window: PE transpose, rho MLP with per-partition bias APs
    (rho_b1/rho_b2), result written into a [2, 6272] SBUF tile; ONE output
    DMA at the end.
"""

import os
import sys

sys.path.insert(0, "/opt/trn_rl_repo")

import numpy as np

last_results = None

def _ensure_axon_profile_hook():
    """The RL container's antenv stub lacks axon_hooks; synthesize it so a
    traced run (trace=True or BASS_TRACE=1) can capture NTFF profiles
    instead of crashing on the missing import. No-op when already present."""
    try:
        import antenv
        import importlib
        try:
            importlib.import_module("antenv.axon_hooks")
            return
        except ImportError:
            pass
        import types
        import trn_agent_boot.trn_boot as tb

        hook = tb._ntff_profile_via_ctypes("/opt/axon/libaxon_pjrt.so")
        mod = types.ModuleType("antenv.axon_hooks")
        mod._hook = hook
        mod.get_axon_ntff_profile_hook = lambda: mod._hook
        mod.set_axon_ntff_profile_hook = lambda h: setattr(mod, "_hook", h)
        sys.modules["antenv.axon_hooks"] = mod
        antenv.axon_hooks = mod
    except Exception:
        pass
  # stash of BassKernelResults when KERNEL_TRACE=1

N_AGENTS = 50000
N_NEIGH = 1600000
D = 64
N_CORES = 8
SEG_T = 128  # segments per window (= PSUM partition dim)
W_PER_CORE = 49
NW_TOT = N_CORES * W_PER_CORE  # 392 windows of 128 segs >= 50000
SEGS_PER_CORE = W_PER_CORE * SEG_T  # 6272
RELU_BATCH = 7  # blocks per batched relu; 7*65 = 455 f32 <= one PSUM bank


def _build_program(B):
    """Build the SPMD bacc program for B blocks (of 128 rows) per window."""
    from concourse import bacc, mybir
    import concourse.tile as tile

    BF16 = mybir.dt.bfloat16
    FP16 = mybir.dt.float16
    F32 = mybir.dt.float32
    Relu = mybir.ActivationFunctionType.Relu
    Ident = mybir.ActivationFunctionType.Identity
    Copy = mybir.ActivationFunctionType.Copy

    NBLK = W_PER_CORE * B
    NCOL = NBLK * 128
    WCOL = B * 128  # columns per window

    nc = bacc.Bacc("TRN2", target_bir_lowering=False, debug=False)
    xta = nc.dram_tensor("xta", [65, NCOL], BF16, kind="ExternalInput").ap()
    qid = nc.dram_tensor("qid", [128, NBLK], F32, kind="ExternalInput").ap()
    w1a = nc.dram_tensor("w1a", [65, 65], BF16, kind="ExternalInput").ap()
    waa = nc.dram_tensor("waa", [65, 64], FP16, kind="ExternalInput").ap()
    wba = nc.dram_tensor("wba", [64, 2], FP16, kind="ExternalInput").ap()
    rb1 = nc.dram_tensor("rb1", [64, 1], F32, kind="ExternalInput").ap()
    rb2 = nc.dram_tensor("rb2", [2, 1], F32, kind="ExternalInput").ap()
    iota = nc.dram_tensor("iota", [128, 128], BF16, kind="ExternalInput").ap()
    iden = nc.dram_tensor("iden", [128, 128], FP16, kind="ExternalInput").ap()
    out = nc.dram_tensor("out", [2, SEGS_PER_CORE], F32, kind="ExternalOutput").ap()

    with tile.TileContext(nc) as tc:
        with (
            tc.tile_pool(name="const", bufs=1) as cpool,
            tc.tile_pool(name="x", bufs=3) as xpool,
            tc.tile_pool(name="h", bufs=4) as hpool,
            tc.tile_pool(name="oh", bufs=8) as ohpool,
            tc.tile_pool(name="rho", bufs=2) as rpool,
            tc.tile_pool(name="psh", bufs=2, space="PSUM") as psh,
            tc.tile_pool(name="pss", bufs=2, space="PSUM") as pss,
            tc.tile_pool(name="pst", bufs=1, space="PSUM") as pst,
            tc.tile_pool(name="psr", bufs=1, space="PSUM") as psr,
            tc.tile_pool(name="pso", bufs=1, space="PSUM") as pso,
        ):
            w1a_t = cpool.tile([65, 65], BF16)
            nc.sync.dma_start(w1a_t[:], w1a[:, :])
            waa_t = cpool.tile([65, 64], FP16)
            nc.sync.dma_start(waa_t[:], waa[:, :])
            wba_t = cpool.tile([64, 2], FP16)
            nc.sync.dma_start(wba_t[:], wba[:, :])
            rb1_t = cpool.tile([64, 1], F32)
            nc.sync.dma_start(rb1_t[:], rb1[:, :])
            rb2_t = cpool.tile([2, 1], F32)
            nc.sync.dma_start(rb2_t[:], rb2[:, :])
            iota_t = cpool.tile([128, 128], BF16)
            nc.sync.dma_start(iota_t[:], iota[:, :])
            iden_t = cpool.tile([128, 128], FP16)
            nc.sync.dma_start(iden_t[:], iden[:, :])
            # all per-block segment ids, loaded once: [128, NBLK] f32
            qall_t = cpool.tile([128, NBLK], F32)
            nc.sync.dma_start(qall_t[:], qid[:, :])
            # final output accumulates here; single DMA at the end
            out_t = cpool.tile([2, SEGS_PER_CORE], F32)

            for w in range(W_PER_CORE):
                # one big DMA per window
                xt = xpool.tile([65, WCOL], BF16)
                nc.sync.dma_start(xt[:], xta[:, WCOL * w : WCOL * (w + 1)])

                s_ps = pss.tile([128, 65], F32)
                b = 0
                while b < B:
                    nb = min(RELU_BATCH, B - b)
                    hp = psh.tile([128, RELU_BATCH * 65], F32)
                    for k in range(nb):
                        nc.tensor.matmul(
                            hp[:, 65 * k : 65 * k + 65],
                            lhsT=xt[:, 128 * (b + k) : 128 * (b + k) + 128],
                            rhs=w1a_t[:],
                            start=True,
                            stop=True,
                        )
                    hs = hpool.tile([128, RELU_BATCH * 65], BF16)
                    nc.scalar.activation(
                        hs[:, : 65 * nb], hp[:, : 65 * nb], Relu
                    )
                    for k in range(nb):
                        oh = ohpool.tile([128, 128], BF16)
                        nc.vector.tensor_scalar(
                            out=oh[:],
                            in0=iota_t[:],
                            scalar1=qall_t[:, B * w + b + k : B * w + b + k + 1],
                            scalar2=None,
                            op0=mybir.AluOpType.is_equal,
                        )
                        nc.tensor.matmul(
                            s_ps[:],
                            lhsT=oh[:],
                            rhs=hs[:, 65 * k : 65 * k + 65],
                            start=(b + k == 0),
                            stop=(b + k == B - 1),
                        )
                    b += nb
                # rho MLP on the window's 128 pooled segments
                s_sb = rpool.tile([128, 65], FP16)
                nc.scalar.activation(s_sb[:], s_ps[:], Copy)
                st_ps = pst.tile([65, 128], FP16)
                nc.tensor.transpose(st_ps[:], s_sb[:], iden_t[:])
                st_sb = rpool.tile([65, 128], FP16)
                nc.vector.tensor_copy(st_sb[:], st_ps[:])
                r_ps = psr.tile([64, 128], F32)
                nc.tensor.matmul(r_ps[:], lhsT=waa_t[:], rhs=st_sb[:], start=True, stop=True)
                r_sb = rpool.tile([64, 128], FP16)
                nc.scalar.activation(r_sb[:], r_ps[:], Relu, bias=rb1_t[:])
                o_ps = pso.tile([2, 128], F32)
                nc.tensor.matmul(o_ps[:], lhsT=wba_t[:], rhs=r_sb[:], start=True, stop=True)
                nc.scalar.activation(
                    out_t[:, SEG_T * w : SEG_T * (w + 1)], o_ps[:], Ident,
                    bias=rb2_t[:],
                )
            nc.sync.dma_start(out[:, :], out_t[:])
    nc.compile()
    return nc


def _host_prep(neighbors, phi_w1, phi_b1, phi_w2, phi_b2,
               rho_w1, rho_b1, rho_w2, rho_b2, segment_ids):
    ids = np.asarray(segment_ids)
    X = np.asarray(neighbors)

    bounds = np.minimum(np.arange(NW_TOT + 1) * SEG_T, N_AGENTS)
    edges = np.searchsorted(ids, bounds)  # row range per window
    rows_w = np.diff(edges)
    B = int(np.ceil(rows_w.max() / 128))

    NBLK = W_PER_CORE * B
    NCOL = NBLK * 128

    import ml_dtypes

    BF = ml_dtypes.bfloat16
    XT = np.ascontiguousarray(X.T).astype(BF)  # [64, N]

    w1a = np.zeros((65, 65), np.float32)
    w1a[0:64, 0:64] = phi_w1
    w1a[64, 0:64] = phi_b1
    w1a[64, 64] = 1.0

    in_maps = []
    consts = dict(
        w1a=w1a.astype(BF),
        waa=np.concatenate(
            [phi_w2 @ rho_w1, (phi_b2 @ rho_w1)[None, :]], 0
        ).astype(np.float16),
        wba=np.asarray(rho_w2).astype(np.float16),
        rb1=np.asarray(rho_b1).reshape(64, 1).astype(np.float32),
        rb2=np.asarray(rho_b2).reshape(2, 1).astype(np.float32),
        iota=np.tile(np.arange(128, dtype=np.float32), (128, 1)).astype(BF),
        iden=np.eye(128, dtype=np.float16),
    )
    for c in range(N_CORES):
        xta = np.zeros((65, NCOL), BF)
        qflat = np.full(NCOL, -1.0, np.float32)
        for wl in range(W_PER_CORE):
            wg = W_PER_CORE * c + wl
            a, e = edges[wg], edges[wg + 1]
            n = e - a
            c0 = wl * B * 128
            xta[0:64, c0 : c0 + n] = XT[:, a:e]
            xta[64, c0 : c0 + n] = 1.0
            qflat[c0 : c0 + n] = (ids[a:e] - SEG_T * wg).astype(np.float32)
        qid = np.ascontiguousarray(qflat.reshape(NBLK, 128).T)
        in_maps.append(dict(xta=xta, qid=qid, **consts))
    return B, in_maps


def kernel(**inputs):
    global last_results
    np_inputs = {k: np.asarray(v) for k, v in inputs.items()}
    B, in_maps = _host_prep(**np_inputs)
    nc = _build_program(B)

    _ensure_axon_profile_hook()
    from concourse.bass_utils import run_bass_kernel_spmd

    trace = bool(os.environ.get("KERNEL_TRACE"))
    res = run_bass_kernel_spmd(nc, in_maps, list(range(N_CORES)), trace=trace)
    if trace:
        last_results = res
    out_t = np.concatenate(
        [res.results[c]["out"] for c in range(N_CORES)], axis=1
    )  # [2, 50176]
    return np.ascontiguousarray(out_t[:, :N_AGENTS].T).astype(np.float32)


# revision 8
# speedup vs baseline: 3.5547x; 1.0226x over previous
"""DeepSet GNN kernel, slot-padded streaming design (design C).

Key idea: pad each segment's neighbor rows into fixed 40-row *slots* so
segment_sum becomes (a) a fixed-stride windowed reduction over h plus (b) a
tiny per-window slot->segment merge matmul. This removes the per-128-row
one-hot builds (DVE) and per-block LDWEIGHTS (PE) of the one-hot design:

  - X^T is packed two-halves-high [128, NCOL/2] (full DMA port width):
    partitions 0:64 = features of "half A" slots, 64:128 = "half B" slots.
  - mm1: ONE full-array matmul per 512-col chunk with the resident
    block-diagonal stationary [[w1,0],[0,w1]] -> h^T [128, 512] in PSUM
    (partitions 0:64 = h of A-rows, 64:128 = h of B-rows).
  - ACT relu (+phi_b1 per-partition bias) -> whole-window hs in SBUF fp16.
  - One DVE windowed tensor_reduce per window over each 40-col slot
    -> slot sums [128, 64].
  - PE transpose -> [64 slots, 128]; two merge matmuls with a DVE-built
    slot->segment one-hot (ONE tensor_scalar per window)
    -> merged [SEG_W, 64] segment sums.
  - rho MLP per window; counts term via a rank-1 matmul with host counts.
"""

import os
import sys

sys.path.insert(0, "/opt/trn_rl_repo")

import numpy as np

last_results = None

def _ensure_axon_profile_hook():
    """The RL container's antenv stub lacks axon_hooks; synthesize it so a
    traced run (trace=True or BASS_TRACE=1) can capture NTFF profiles
    instead of crashing on the missing import. No-op when already present."""
    try:
        import antenv
        import importlib
        try:
            importlib.import_module("antenv.axon_hooks")
            return
        except ImportError:
            pass
        import types
        import trn_agent_boot.trn_boot as tb

        hook = tb._ntff_profile_via_ctypes("/opt/axon/libaxon_pjrt.so")
        mod = types.ModuleType("antenv.axon_hooks")
        mod._hook = hook
        mod.get_axon_ntff_profile_hook = lambda: mod._hook
        mod.set_axon_ntff_profile_hook = lambda h: setattr(mod, "_hook", h)
        sys.modules["antenv.axon_hooks"] = mod
        antenv.axon_hooks = mod
    except Exception:
        pass


N_AGENTS = 50000
N_CORES = 8
SEG_PC = N_AGENTS // N_CORES  # 6250 segments per core
P_SLOT = 40  # rows per slot
SLOT_W = 128  # slots per window (64 per half)
HALF_SLOTS = 64
WCOL = HALF_SLOTS * P_SLOT  # 2560 columns per window tile
CHUNK = 512  # cols per matmul chunk (one PSUM bank)
CHUNKS_PER_WIN = WCOL // CHUNK  # 5


def _build_program(SEG_W, NW):
    from concourse import bacc, mybir
    import concourse.tile as tile

    FP16 = mybir.dt.float16
    F32 = mybir.dt.float32
    Relu = mybir.ActivationFunctionType.Relu
    Ident = mybir.ActivationFunctionType.Identity
    Copy = mybir.ActivationFunctionType.Copy
    AX = mybir.AxisListType.X
    ADD = mybir.AluOpType.add
    ISEQ = mybir.AluOpType.is_equal

    NCOL2 = NW * WCOL
    OUTW = NW * SEG_W
    CPS = CHUNK // P_SLOT  # 12.8 -> NOT integral; hs is sliced by cols instead

    nc = bacc.Bacc("TRN2", target_bir_lowering=False, debug=False)
    xta = nc.dram_tensor("xta", [128, NCOL2], FP16, kind="ExternalInput").ap()
    w1bd = nc.dram_tensor("w1bd", [128, 128], FP16, kind="ExternalInput").ap()
    b1d = nc.dram_tensor("b1d", [128, 1], F32, kind="ExternalInput").ap()
    waa = nc.dram_tensor("waa", [64, 64], FP16, kind="ExternalInput").ap()
    vb2 = nc.dram_tensor("vb2", [1, 64], FP16, kind="ExternalInput").ap()
    wba = nc.dram_tensor("wba", [64, 2], FP16, kind="ExternalInput").ap()
    rb1 = nc.dram_tensor("rb1", [64, 1], F32, kind="ExternalInput").ap()
    rb2 = nc.dram_tensor("rb2", [2, 1], F32, kind="ExternalInput").ap()
    iotas = nc.dram_tensor("iotas", [SLOT_W, SEG_W], FP16, kind="ExternalInput").ap()
    sos = nc.dram_tensor("sos", [SLOT_W, NW], F32, kind="ExternalInput").ap()
    cntr = nc.dram_tensor("cntr", [1, OUTW], FP16, kind="ExternalInput").ap()
    idenf = nc.dram_tensor("idenf", [128, 128], F32, kind="ExternalInput").ap()
    idenh = nc.dram_tensor("idenh", [128, 128], FP16, kind="ExternalInput").ap()
    out = nc.dram_tensor("out", [2, OUTW], F32, kind="ExternalOutput").ap()

    with tile.TileContext(nc) as tc:
        with (
            tc.tile_pool(name="const", bufs=1) as cpool,
            tc.tile_pool(name="x", bufs=4) as xpool,
            tc.tile_pool(name="h", bufs=3) as hpool,
            tc.tile_pool(name="ss", bufs=5) as spool,
            tc.tile_pool(name="mg", bufs=8) as mpool,
            tc.tile_pool(name="rho", bufs=3) as rpool,
            tc.tile_pool(name="psh", bufs=4, space="PSUM") as psh,
            tc.tile_pool(name="pst1", bufs=1, space="PSUM") as pst1,
            tc.tile_pool(name="pss", bufs=1, space="PSUM") as pss,
            tc.tile_pool(name="pse", bufs=2, space="PSUM") as pse,
        ):
            w1bd_t = cpool.tile([128, 128], FP16)
            nc.sync.dma_start(w1bd_t[:], w1bd[:, :])
            b1d_t = cpool.tile([128, 1], F32)
            nc.sync.dma_start(b1d_t[:], b1d[:, :])
            waa_t = cpool.tile([64, 64], FP16)
            nc.sync.dma_start(waa_t[:], waa[:, :])
            vb2_t = cpool.tile([1, 64], FP16)
            nc.sync.dma_start(vb2_t[:], vb2[:, :])
            wba_t = cpool.tile([64, 2], FP16)
            nc.sync.dma_start(wba_t[:], wba[:, :])
            rb1_t = cpool.tile([64, 1], F32)
            nc.sync.dma_start(rb1_t[:], rb1[:, :])
            rb2_t = cpool.tile([2, 1], F32)
            nc.sync.dma_start(rb2_t[:], rb2[:, :])
            iotas_t = cpool.tile([SLOT_W, SEG_W], FP16)
            nc.sync.dma_start(iotas_t[:], iotas[:, :])
            sosa_t = cpool.tile([HALF_SLOTS, NW], F32)
            nc.sync.dma_start(sosa_t[:], sos[0:HALF_SLOTS, :])
            sosb_t = cpool.tile([HALF_SLOTS, NW], F32)
            nc.sync.dma_start(sosb_t[:], sos[HALF_SLOTS:SLOT_W, :])
            cntr_t = cpool.tile([1, OUTW], FP16)
            nc.sync.dma_start(cntr_t[:], cntr[:, :])
            idenf_t = cpool.tile([128, 128], F32)
            nc.sync.dma_start(idenf_t[:], idenf[:, :])
            idenh_t = cpool.tile([128, 128], FP16)
            nc.sync.dma_start(idenh_t[:], idenh[:, :])
            out_t = cpool.tile([2, OUTW], F32)

            pend = {}

            def front(w):
                xt = xpool.tile([128, WCOL], FP16)
                nc.sync.dma_start(xt[:], xta[:, WCOL * w : WCOL * (w + 1)])
                hs = hpool.tile([128, HALF_SLOTS, P_SLOT], FP16)
                for k in range(CHUNKS_PER_WIN):
                    hp = psh.tile([128, CHUNK], F32)
                    nc.tensor.matmul(
                        hp[:, :],
                        lhsT=w1bd_t[:],
                        rhs=xt[:, CHUNK * k : CHUNK * (k + 1)],
                        start=True,
                        stop=True,
                    )
                    hsf = hs[:, :, :].rearrange("p a b -> p (a b)")
                    nc.scalar.activation(
                        hsf[:, CHUNK * k : CHUNK * (k + 1)], hp[:, :], Relu,
                        bias=b1d_t[:],
                    )
                t1 = hpool.tile([128, HALF_SLOTS, 20], FP16, tag="t1")
                nc.vector.tensor_tensor(
                    out=t1[:, :, :], in0=hs[:, :, 0:20], in1=hs[:, :, 20:40], op=ADD
                )
                t2 = hpool.tile([128, HALF_SLOTS, 10], FP16, tag="t2")
                nc.vector.tensor_tensor(
                    out=t2[:, :, :], in0=t1[:, :, 0:10], in1=t1[:, :, 10:20], op=ADD
                )
                ssum = spool.tile([128, HALF_SLOTS], F32)
                nc.vector.tensor_reduce(ssum[:], t2[:, :, :], axis=AX, op=ADD)
                # merge one-hots on the otherwise-idle GpSimd engine
                mga = mpool.tile([HALF_SLOTS, SEG_W], FP16)
                nc.vector.tensor_scalar(
                    out=mga[:], in0=iotas_t[0:HALF_SLOTS, :],
                    scalar1=sosa_t[:, w : w + 1],
                    scalar2=None, op0=ISEQ,
                )
                mgb = mpool.tile([HALF_SLOTS, SEG_W], FP16)
                nc.vector.tensor_scalar(
                    out=mgb[:], in0=iotas_t[0:HALF_SLOTS, :],
                    scalar1=sosb_t[:, w : w + 1],
                    scalar2=None, op0=ISEQ,
                )
                pend[w] = (ssum, mga, mgb)

            def back_merge(w):
                ssum, mga, mgb = pend.pop(w)
                st1 = pst1.tile([HALF_SLOTS, 128], F32)
                nc.tensor.transpose(st1[:], ssum[:], idenf_t[:])
                stc = spool.tile([HALF_SLOTS, 128], FP16)
                nc.vector.tensor_copy(stc[:], st1[:])
                merged = pss.tile([64, SEG_W], F32)
                nc.tensor.matmul(merged[:], lhsT=stc[:, 0:64], rhs=mga[:], start=True, stop=False)
                nc.tensor.matmul(merged[:], lhsT=stc[:, 64:128], rhs=mgb[:], start=False, stop=True)
                return merged

            def back_pair(w):
                m0 = back_merge(w)
                m1 = back_merge(w + 1)
                st2_sb = rpool.tile([64, 2 * SEG_W], FP16)
                nc.vector.tensor_copy(st2_sb[:, 0:SEG_W], m0[:])
                nc.vector.tensor_copy(st2_sb[:, SEG_W : 2 * SEG_W], m1[:])
                r_ps = pse.tile([64, 2 * SEG_W], F32, tag="epi")
                nc.tensor.matmul(r_ps[:], lhsT=waa_t[:], rhs=st2_sb[:], start=True, stop=False)
                nc.tensor.matmul(
                    r_ps[:], lhsT=vb2_t[:],
                    rhs=cntr_t[:, SEG_W * w : SEG_W * (w + 2)],
                    start=False, stop=True,
                )
                r_sb = rpool.tile([64, 2 * SEG_W], FP16)
                nc.scalar.activation(r_sb[:], r_ps[:], Relu, bias=rb1_t[:])
                o_ps = pse.tile([2, 2 * SEG_W], F32, tag="epi")
                nc.tensor.matmul(o_ps[:], lhsT=wba_t[:], rhs=r_sb[:], start=True, stop=True)
                nc.scalar.activation(
                    out_t[:, SEG_W * w : SEG_W * (w + 2)], o_ps[:], Ident, bias=rb2_t[:]
                )

            DELAY = 4
            assert NW % 2 == 0
            for w in range(NW):
                front(w)
                if w >= DELAY + 1 and (w - DELAY) % 2 == 1:
                    back_pair(w - DELAY - 1)
            for w in range(NW - DELAY, NW, 2):
                back_pair(w)
            nc.sync.dma_start(out[:, :], out_t[:])
    nc.compile()
    return nc


def _host_prep(neighbors, phi_w1, phi_b1, phi_w2, phi_b2,
               rho_w1, rho_b1, rho_w2, rho_b2, segment_ids):
    ids = np.asarray(segment_ids)
    X = np.asarray(neighbors)
    r0 = np.searchsorted(ids, np.arange(N_AGENTS + 1))
    d = np.diff(r0)  # rows per segment
    kslots = -(-d // P_SLOT)  # ceil; 0 for empty segments

    SEG_W = None
    for cand in (108, 104, 100, 96, 88, 80):
        ok = True
        for c in range(N_CORES):
            ks = kslots[SEG_PC * c : SEG_PC * (c + 1)]
            nw = -(-SEG_PC // cand)
            pad = np.zeros(nw * cand, np.int64)
            pad[: SEG_PC] = ks
            if pad.reshape(nw, cand).sum(1).max() > SLOT_W:
                ok = False
                break
        if ok:
            SEG_W = cand
            break
    assert SEG_W is not None, "no SEG_W candidate fits the slot budget"
    NW = -(-SEG_PC // SEG_W)
    OUTW = NW * SEG_W

    XT = np.ascontiguousarray(X.T).astype(np.float16)  # [64, N]

    w1bd = np.zeros((128, 128), np.float32)
    w1bd[0:64, 0:64] = phi_w1
    w1bd[64:128, 64:128] = phi_w1

    consts = dict(
        w1bd=w1bd.astype(np.float16),
        b1d=np.concatenate([phi_b1, phi_b1], 0).reshape(128, 1).astype(np.float32),
        waa=(phi_w2 @ rho_w1).astype(np.float16),
        vb2=np.asarray(phi_b2 @ rho_w1).reshape(1, 64).astype(np.float16),
        wba=np.asarray(rho_w2).astype(np.float16),
        rb1=np.asarray(rho_b1).reshape(64, 1).astype(np.float32),
        rb2=np.asarray(rho_b2).reshape(2, 1).astype(np.float32),
        iotas=np.tile(np.arange(SEG_W, dtype=np.float32), (SLOT_W, 1)).astype(np.float16),
        idenf=np.eye(128, dtype=np.float32),
        idenh=np.eye(128, dtype=np.float16),
    )

    in_maps = []
    for c in range(N_CORES):
        sos = np.full((SLOT_W, NW), -1.0, np.float32)
        cnt = np.zeros((SEG_W, NW), np.float32)
        colmap = np.full((NW, 2, WCOL), -1, np.int64)
        for w in range(NW):
            lo = SEG_PC * c + SEG_W * w
            hi = min(lo + SEG_W, SEG_PC * (c + 1))
            cnt[0 : hi - lo, w] = d[lo:hi]
            si = 0
            for s in range(lo, hi):
                for j in range(kslots[s]):
                    ln = min(P_SLOT, d[s] - P_SLOT * j)
                    half, jj = divmod(si, HALF_SLOTS)
                    c0 = jj * P_SLOT
                    colmap[w, half, c0 : c0 + ln] = r0[s] + P_SLOT * j + np.arange(ln)
                    sos[si, w] = s - lo
                    si += 1
        xta = np.zeros((128, NW * WCOL), np.float16)
        for half in range(2):
            cm = colmap[:, half, :].reshape(-1)
            g = XT[:, np.clip(cm, 0, None)]
            g[:, cm < 0] = 0
            xta[64 * half : 64 * half + 64, :] = g
        in_maps.append(dict(
            xta=xta,
            sos=sos,
            cntr=np.ascontiguousarray(cnt.T.reshape(1, OUTW)).astype(np.float16),
            **consts,
        ))
    return SEG_W, NW, in_maps


def kernel(**inputs):
    global last_results
    np_inputs = {k: np.asarray(v) for k, v in inputs.items()}
    SEG_W, NW, in_maps = _host_prep(**np_inputs)
    nc = _build_program(SEG_W, NW)

    _ensure_axon_profile_hook()
    from concourse.bass_utils import run_bass_kernel_spmd

    trace = bool(os.environ.get("KERNEL_TRACE"))
    res = run_bass_kernel_spmd(nc, in_maps, list(range(N_CORES)), trace=trace)
    if trace:
        last_results = res
    cols = []
    for c in range(N_CORES):
        cols.append(res.results[c]["out"][:, :SEG_PC])
    out_t = np.concatenate(cols, 1)  # [2, 50000]
    return np.ascontiguousarray(out_t.T).astype(np.float32)
